# revision 2
# baseline (speedup 1.0000x reference)
"""Two-layer GAT on 8 TRN2 cores — v2: batched dma_gather edge gathers.

vs v1 (per-slot-column indirect_dma_start, 8.6ns/row on one SWDGE context):
per-edge source rows are fetched with InstDMAGatherAnt (<=1024 int16 indices
per instruction) round-robined over 4 SWDGE queue contexts (~2.3ns/row).
int16 indices force 4 table chunks of 25088 rows; chunk membership = core
pair owning the node (table order = AllGather order pos2), chosen by a
balanced coloring so per-(tile,chunk) ELL widths stay small. One shared
wrapped-index table serves both layers. Table rows are 256B (dma_gather
minimum transfer); row 0 of each chunk is a -inf dummy for padding slots.
"""

import os
import numpy as np
from contextlib import ExitStack

import concourse.bass as bass
import concourse.bacc as bacc
import concourse.tile as tile
from concourse import mybir
from concourse.bass import AP
from concourse.bass_utils import run_bass_kernel_spmd

P = 128
NCORE = 8
NCHUNK = 4
CH = 25088
CHR = CH + 1
NEG = 0.2
EPS = 1e-16
NEGINF = -1.0e30
F32 = mybir.dt.float32
I16 = mybir.dt.int16
ROWF = 64          # f32 per table row (256B)

AS0X, AS1X, AD0, AD1 = 0, 16, 32, 40
W1BLK, W2EXT, B2, B1, IDENT = 48, 176, 182, 186, 187
CW = 320


def _v(t_ap: AP, off: int, dims) -> AP:
    return AP(t_ap.tensor, t_ap.offset + off,
              [list(t_ap.ap[0])] + [list(d) for d in dims])


def _dv(handle, off: int, dims) -> AP:
    base = handle[:]
    return AP(base.tensor, off, [list(d) for d in dims])


def _color_nodes(src, dst, order, tgrp, T, N, N_pad, seed=0):
    rng = np.random.default_rng(seed)
    eo = np.argsort(src, kind="stable")
    dsts_of = dst[eo]
    csr = np.zeros(N + 1, np.int64)
    csr[1:] = np.cumsum(np.bincount(src, minlength=N))
    outdeg = np.diff(csr)
    cnt = np.zeros((N, NCHUNK), np.int32)
    quota = np.full((T, NCHUNK), 2 * P, np.int32)
    color = np.full(N_pad, -1, np.int8)
    BIG = np.iinfo(np.int64).max
    for n in np.argsort(-outdeg, kind="stable"):
        tg = tgrp[n]
        rows = dsts_of[csr[n]:csr[n + 1]]
        if len(rows):
            load = cnt[rows].astype(np.int64)
            score = (load * load + load).sum(axis=0)
        else:
            score = rng.integers(0, 4, NCHUNK)
        score = np.where(quota[tg] > 0, score, BIG)
        c = int(np.argmin(score))
        color[n] = c
        quota[tg, c] -= 1
        if len(rows):
            cnt[rows, c] += 1
    for n in range(N, N_pad):
        tg = tgrp[n]
        c = int(np.argmax(quota[tg]))
        color[n] = c
        quota[tg, c] -= 1
    return color, cnt


def _plan(src, dst, N):
    E = len(src)
    deg = np.bincount(dst, minlength=N).astype(np.int64)
    T = int(np.ceil(N / (P * NCORE)))
    NT = T * NCORE
    N_pad = NT * P
    assert N_pad == NCHUNK * CH
    order = np.concatenate([np.argsort(-deg, kind="stable"),
                            np.arange(N, N_pad)])
    inv = np.empty(N_pad, np.int64)
    inv[order] = np.arange(N_pad)
    tgrp = inv // (NCORE * P)

    color, cnt = _color_nodes(src, dst, order, tgrp, T, N, N_pad)

    core = np.empty(N_pad, np.int64)
    part = np.empty(N_pad, np.int64)
    ltile = np.empty(N_pad, np.int64)
    deg_pad = np.concatenate([deg, np.zeros(N_pad - N, np.int64)])
    for t in range(T):
        g_nodes = order[t * NCORE * P:(t + 1) * NCORE * P]
        for c in range(NCHUNK):
            m = g_nodes[color[g_nodes] == c]
            assert len(m) == 2 * P
            m = m[np.argsort(-deg_pad[m], kind="stable")]
            core[m[0::2]] = 2 * c
            core[m[1::2]] = 2 * c + 1
            part[m[0::2]] = np.arange(P)
            part[m[1::2]] = np.arange(P)
            ltile[m] = t
    pos2 = core * (T * P) + ltile * P + part

    D = np.zeros((T, NCHUNK), np.int64)
    for t in range(T):
        g = order[t * NCORE * P:(t + 1) * NCORE * P]
        g = g[g < N]
        if len(g):
            D[t] = cnt[g].max(axis=0)
    S = int(D.sum())
    colbase = np.zeros((T, NCHUNK), np.int64)
    colbase.reshape(-1)[1:] = D.reshape(-1).cumsum()[:-1]

    si = np.zeros((NCORE, P, S), np.int16)
    eorder = np.argsort(dst, kind="stable")
    ds = dst[eorder]
    ss = src[eorder]
    ccol = color[ss].astype(np.int64)
    relidx = (pos2[ss] - ccol * CH + 1).astype(np.int16)
    key = ds * 4 + ccol
    korder = np.argsort(key, kind="stable")
    kk = key[korder]
    j = np.arange(E) - np.concatenate(
        [[0], np.cumsum(np.bincount(kk, minlength=4 * N))])[kk]
    si[core[ds[korder]], part[ds[korder]],
       colbase[ltile[ds[korder]], ccol[korder]] + j] = relidx[korder]

    # wrapped idx layout per (t, c) rect, replicated across 8 groups
    siw = np.zeros((NCORE, P, S * 8), np.int16)
    for t in range(T):
        for c in range(NCHUNK):
            dct = int(D[t, c])
            if dct == 0:
                continue
            cb = int(colbase[t, c])
            rect = si[:, :, cb:cb + dct]                  # [8, P, D]
            lst = rect.transpose(0, 2, 1).reshape(NCORE, -1)   # col-major
            w = lst.reshape(NCORE, -1, 16).transpose(0, 2, 1)  # [8,16,NI/16]
            siw[:, :, cb * 8:(cb + dct) * 8] = np.tile(w, (1, 8, 1))

    dstid = np.zeros((NCORE, P, T), np.int64)
    dstid[core, part, ltile] = np.arange(N_pad)
    perm_tab = np.empty(N_pad, np.int64)
    perm_tab[pos2] = np.arange(N_pad)
    return dict(T=T, N_pad=N_pad, S=S, D=D, colbase=colbase, pos2=pos2,
                perm_tab=perm_tab, siw=siw, dstid=dstid)


def _consts(W1, att_src1, att_dst1, b1, W2, att_src2, att_dst2, b2):
    W1r = W1.reshape(2, 8, 16)
    As = np.einsum("khc,hc->kh", W1r, att_src1)
    Ad = np.einsum("khc,hc->kh", W1r, att_dst1)
    c = np.zeros((P, CW), np.float32)
    c[:, AS0X:AS0X + 8] = As[0]
    c[:, AS0X + 8] = 1.0
    c[:, AS1X:AS1X + 8] = As[1]
    c[:, AS1X + 9] = 1.0
    c[:, AD0:AD0 + 8] = Ad[0]
    c[:, AD1:AD1 + 8] = Ad[1]
    w1blk = np.zeros((16, 128), np.float32)
    for k in range(2):
        for h in range(8):
            w1blk[k * 8 + h, h * 16:(h + 1) * 16] = W1r[k, h]
    c[:16, W1BLK:W1BLK + 128] = w1blk
    c[:, W2EXT:W2EXT + 4] = W2
    c[:, W2EXT + 4] = W2 @ att_src2[0]
    c[:, W2EXT + 5] = W2 @ att_dst2[0]
    c[:, B2:B2 + 4] = b2
    c[:, B1] = b1
    c[:, IDENT:IDENT + 128] = np.eye(P, dtype=np.float32)
    dum = np.zeros((1, 24), np.float32)
    dum[0, 0:8] = NEGINF
    dum[0, 20] = NEGINF
    return c, dum


def _build(T, S, D, colbase, N_pad):
    nc = bacc.Bacc("TRN2", target_bir_lowering=False, num_swdge_queues=4)
    xin = nc.declare_dram_parameter("xpad", [N_pad, 2], F32, isOutput=False)
    siw_in = nc.declare_dram_parameter("siw", [P, S * 8], I16, isOutput=False)
    xdin = nc.declare_dram_parameter("xd", [P, T * 2], F32, isOutput=False)
    cin = nc.declare_dram_parameter("consts", [P, CW], F32, isOutput=False)
    din = nc.declare_dram_parameter("dum", [1, 24], F32, isOutput=False)
    oext = nc.declare_dram_parameter("out", [T * P, 4], F32, isOutput=True)

    z1tab = nc.dram_tensor("z1tab", [NCHUNK * CHR, ROWF], F32)
    t2tab = nc.dram_tensor("t2tab", [NCHUNK * CHR, ROWF], F32)
    t2c = nc.dram_tensor("t2c", [T * P, 8], F32)
    t2all = nc.dram_tensor("t2all", [N_pad, 8], F32, addr_space="Shared")

    J = N_pad // P
    ACT = mybir.ActivationFunctionType
    ALU = mybir.AluOpType
    qctr = [0]

    def gathers(pool, tag, tab, t):
        """Issue the 4-chunk gathers of tile t into one [P, S_t*ROWF] tile."""
        St = int(D[t].sum())
        g = pool.tile([P, St * ROWF], F32, tag=tag)
        cb0 = int(colbase[t, 0])
        for c in range(NCHUNK):
            dct = int(D[t, c])
            if dct == 0:
                continue
            cb = int(colbase[t, c])
            off = 0
            while off < dct:
                dd = min(8, dct - off)
                ni = P * dd
                nc.gpsimd.dma_gather(
                    _v(g[:], (cb - cb0 + off) * ROWF,
                       [[ROWF, dd], [1, ROWF]]),
                    _dv(tab, c * CHR * ROWF, [[ROWF, CHR], [1, ROWF]]),
                    siw_sb[:, (cb + off) * 8:(cb + off + dd) * 8],
                    ni, ni, ROWF,
                    queue_num=qctr[0] % 4,
                )
                qctr[0] += 1
                off += dd
        return g, St

    with tile.TileContext(nc) as tc, ExitStack() as ctx:
        persist = ctx.enter_context(tc.tile_pool(name="persist", bufs=1))
        build = ctx.enter_context(tc.tile_pool(name="build", bufs=1))
        gp = ctx.enter_context(tc.tile_pool(name="gath", bufs=4))
        wk = ctx.enter_context(tc.tile_pool(name="work", bufs=2))
        sm = ctx.enter_context(tc.tile_pool(name="small", bufs=3))
        pp = ctx.enter_context(tc.tile_pool(name="psA", bufs=2, space="PSUM"))
        pq = ctx.enter_context(tc.tile_pool(name="psB", bufs=2, space="PSUM"))

        csb = persist.tile([P, CW], F32)
        nc.sync.dma_start(out=csb[:], in_=cin[:])
        dsb = persist.tile([1, 24], F32)
        nc.sync.dma_start(out=dsb[:], in_=din[:])
        siw_sb = persist.tile([P, S * 8], I16)
        nc.sync.dma_start(out=siw_sb[:], in_=siw_in[:])
        h3eS = persist.tile([P, T * 6], F32)
        adstE = persist.tile([P, T * 8], F32)
        x_sb = persist.tile([P, J * 2], F32)
        nc.sync.dma_start(out=x_sb[:],
                          in_=xin[:].rearrange("(p j) c -> p (j c)", p=P))
        tc.strict_bb_all_engine_barrier()

        # ---- prologue: Z1 table in pos2 order (node r = p*J + j) ----
        nch = max(1, (J + 97) // 98)
        jc = (J + nch - 1) // nch
        for c0 in range(0, J, jc):
            jn = min(jc, J - c0)
            z1_sb = build.tile([P, jc * 16], F32, tag="zb")
            tt = build.tile([P, jc * 16], F32, tag="tb")
            x0b = _v(x_sb[:], c0 * 2, [[2, jn], [0, 16]])
            x1b = _v(x_sb[:], c0 * 2 + 1, [[2, jn], [0, 16]])
            as0b = _v(csb[:], AS0X, [[0, jn], [1, 16]])
            as1b = _v(csb[:], AS1X, [[0, jn], [1, 16]])
            z3 = _v(z1_sb[:], 0, [[16, jn], [1, 16]])
            t3 = _v(tt[:], 0, [[16, jn], [1, 16]])
            nc.vector.tensor_tensor(out=z3, in0=x0b, in1=as0b, op=ALU.mult)
            nc.vector.tensor_tensor(out=t3, in0=x1b, in1=as1b, op=ALU.mult)
            nc.vector.tensor_tensor(out=z3, in0=z3, in1=t3, op=ALU.add)
            # rows (1+c) + p*J + c0 + j for p in chunk-quadrant c
            for c in range(NCHUNK):
                nc.sync.dma_start(
                    out=_dv(z1tab,
                            ((1 + c * CHR - c * CH) + 32 * c * J + c0) * ROWF,
                            [[J * ROWF, 32], [ROWF, jn], [1, 16]]),
                    in_=z1_sb[32 * c:32 * (c + 1), 0:jn * 16])
        for c in range(NCHUNK):
            nc.sync.dma_start(out=_dv(z1tab, c * CHR * ROWF, [[1, 1], [1, 16]]),
                              in_=dsb[0:1, 0:16])
            nc.sync.dma_start(out=_dv(t2tab, c * CHR * ROWF, [[1, 1], [1, 8]]),
                              in_=dsb[0:1, 16:24])

        xd = persist.tile([P, T * 2], F32)
        nc.sync.dma_start(out=xd[:], in_=xdin[:])
        ttd = build.tile([P, T * 8], F32)
        nc.vector.tensor_tensor(
            out=adstE[:].rearrange("p (t h) -> p t h", h=8),
            in0=_v(xd[:], 0, [[2, T], [0, 8]]),
            in1=_v(csb[:], AD0, [[0, T], [1, 8]]), op=ALU.mult)
        nc.vector.tensor_tensor(
            out=ttd[:].rearrange("p (t h) -> p t h", h=8),
            in0=_v(xd[:], 1, [[2, T], [0, 8]]),
            in1=_v(csb[:], AD1, [[0, T], [1, 8]]), op=ALU.mult)
        nc.vector.tensor_tensor(out=adstE[:], in0=adstE[:], in1=ttd[:],
                                op=ALU.add)
        tc.strict_bb_all_engine_barrier()

        # ---- layer 1 ----
        def lrelu_exp(dst_t, src_t, n):
            tmp = wk.tile([P, n], F32, tag="lrtmp")
            nc.scalar.activation(out=tmp[:], in_=src_t, func=ACT.Prelu,
                                 alpha=NEG)
            nc.scalar.activation(out=dst_t, in_=tmp[:], func=ACT.Exp)

        GnB = persist.tile([P, 16 * 4], F32)
        for t in range(T):
            g, St = gathers(gp, "g1", z1tab, t)
            n8 = 8 * St
            e = wk.tile([P, n8], F32, tag="e1")
            nc.vector.tensor_tensor(
                out=_v(e[:], 0, [[St, 8], [1, St]]),
                in0=_v(g[:], 0, [[1, 8], [ROWF, St]]),
                in1=_v(adstE[:], t * 8, [[1, 8], [0, St]]),
                op=ALU.add)
            ex = wk.tile([P, n8], F32, tag="ex1")
            lrelu_exp(ex[:], e[:], n8)
            s = sm.tile([P, 8], F32, tag="s1")
            nc.vector.tensor_reduce(
                out=s[:], in_=ex[:].rearrange("p (a j) -> p a j", j=St),
                axis=mybir.AxisListType.X, op=ALU.add)
            rs = sm.tile([P, 8], F32, tag="rs1")
            nc.vector.tensor_scalar_add(rs[:], s[:], EPS)
            nc.vector.reciprocal(rs[:], rs[:])
            prod = wk.tile([P, 2 * n8], F32, tag="pr1")
            nc.vector.tensor_tensor(
                out=_v(prod[:], 0, [[n8, 2], [St, 8], [1, St]]),
                in0=_v(ex[:], 0, [[0, 2], [St, 8], [1, St]]),
                in1=_v(g[:], 8, [[1, 2], [0, 8], [ROWF, St]]),
                op=ALU.mult)
            G = sm.tile([P, 16], F32, tag="G1")
            nc.vector.tensor_reduce(
                out=G[:], in_=prod[:].rearrange("p (a j) -> p a j", j=St),
                axis=mybir.AxisListType.X, op=ALU.add)
            nc.vector.tensor_tensor(
                out=_v(GnB[:], (t % 4) * 16, [[8, 2], [1, 8]]),
                in0=G[:].rearrange("p (k h) -> p k h", k=2),
                in1=_v(rs[:], 0, [[0, 2], [1, 8]]),
                op=ALU.mult)
            if t % 4 == 3 or t == T - 1:
                hn = t % 4 + 1
                t0 = t - hn + 1
                GnT = sm.tile([16, 4 * 128], F32, tag="GnT")
                for u in range(hn):
                    pt = pp.tile([P, P], F32, tag="pt")
                    nc.tensor.transpose(
                        out=pt[0:16, :],
                        in_=_v(GnB[:], u * 16, [[8, 2], [1, 8]]),
                        identity=csb[:, IDENT:IDENT + 128])
                    nc.scalar.copy(out=GnT[0:16, u * 128:(u + 1) * 128],
                                   in_=pt[0:16, :])
                o1p = pq.tile([P, 512], F32, tag="o1p")
                nc.tensor.matmul(
                    out=o1p[:, 0:hn * 128],
                    lhsT=csb[0:16, W1BLK:W1BLK + 128],
                    rhs=GnT[0:16, 0:hn * 128],
                    start=True, stop=True)
                h2T = wk.tile([P, 512], F32, tag="h2T")
                nc.scalar.activation(
                    out=h2T[:, 0:hn * 128], in_=o1p[:, 0:hn * 128],
                    func=ACT.Relu, bias=csb[:, B1:B1 + 1], scale=1.0)
                h3p = pq.tile([P, 32], F32, tag="h3p")
                for u in range(hn):
                    nc.tensor.matmul(
                        out=h3p[:, u * 8:u * 8 + 6],
                        lhsT=h2T[:, u * 128:(u + 1) * 128],
                        rhs=csb[:, W2EXT:W2EXT + 6],
                        start=True, stop=True)
                nc.vector.tensor_copy(
                    out=_v(h3eS[:], t0 * 6, [[6, hn], [1, 6]]),
                    in_=_v(h3p[:], 0, [[8, hn], [1, 6]]))

        # ---- share t2: compact write, AllGather, expand to 256B rows ----
        nc.sync.dma_start(
            out=_dv(t2c, 0, [[8, P], [P * 8, T], [1, 6]]),
            in_=_v(h3eS[:], 0, [[6, T], [1, 6]]))
        tc.strict_bb_all_engine_barrier()
        nc.gpsimd.collective_compute(
            "AllGather", ALU.bypass,
            replica_groups=[list(range(NCORE))],
            ins=[t2c[:]], outs=[t2all[:]])
        tc.strict_bb_all_engine_barrier()
        for c in range(NCHUNK):
            eng = [nc.sync, nc.scalar][c % 2]
            eng.dma_start(
                out=_dv(t2tab, (c * CHR + 1) * ROWF,
                        [[ROWF, CH], [1, 8]]),
                in_=_dv(t2all, c * CH * 8, [[8, CH], [1, 8]]))
        tc.strict_bb_all_engine_barrier()

        # ---- layer 2 ----
        o2B = persist.tile([P, 16], F32)
        for t in range(T):
            g2, St = gathers(gp, "g2", t2tab, t)
            e2 = wk.tile([P, St], F32, tag="e2")
            nc.vector.tensor_tensor(
                out=_v(e2[:], 0, [[1, St]]),
                in0=_v(g2[:], 4, [[ROWF, St]]),
                in1=_v(h3eS[:], t * 6 + 5, [[0, St]]),
                op=ALU.add)
            ex2 = wk.tile([P, St], F32, tag="ex2")
            lrelu_exp(ex2[:], e2[:], St)
            s2 = sm.tile([P, 1], F32, tag="s2")
            nc.vector.tensor_reduce(
                out=s2[:], in_=ex2[:].rearrange("p (a j) -> p a j", j=St),
                axis=mybir.AxisListType.X, op=ALU.add)
            rs2 = sm.tile([P, 1], F32, tag="rs2")
            nc.vector.tensor_scalar_add(rs2[:], s2[:], EPS)
            nc.vector.reciprocal(rs2[:], rs2[:])
            prod2 = wk.tile([P, 4 * St], F32, tag="pr2")
            nc.vector.tensor_tensor(
                out=_v(prod2[:], 0, [[St, 4], [1, St]]),
                in0=_v(ex2[:], 0, [[0, 4], [1, St]]),
                in1=_v(g2[:], 0, [[1, 4], [ROWF, St]]),
                op=ALU.mult)
            M2 = sm.tile([P, 4], F32, tag="M2")
            nc.vector.tensor_reduce(
                out=M2[:], in_=prod2[:].rearrange("p (a j) -> p a j", j=St),
                axis=mybir.AxisListType.X, op=ALU.add)
            nc.vector.tensor_tensor(
                out=_v(o2B[:], (t % 4) * 4, [[1, 4]]),
                in0=M2[:],
                in1=_v(rs2[:], 0, [[0, 4]]),
                op=ALU.mult)
            if t % 4 == 3 or t == T - 1:
                hn = t % 4 + 1
                t0 = t - hn + 1
                o2 = sm.tile([P, 4 * hn], F32, tag="o2")
                nc.vector.tensor_tensor(
                    out=o2[:].rearrange("p (t c) -> p t c", c=4),
                    in0=_v(o2B[:], 0, [[4, hn], [1, 4]]),
                    in1=_v(csb[:], B2, [[0, hn], [1, 4]]),
                    op=ALU.add)
                mx = sm.tile([P, hn], F32, tag="mx")
                nc.vector.tensor_reduce(
                    out=mx[:], in_=o2[:].rearrange("p (t c) -> p t c", c=4),
                    axis=mybir.AxisListType.X, op=ALU.max)
                z = sm.tile([P, 4 * hn], F32, tag="z")
                nc.vector.tensor_tensor(
                    out=z[:].rearrange("p (t c) -> p t c", c=4),
                    in0=o2[:].rearrange("p (t c) -> p t c", c=4),
                    in1=_v(mx[:], 0, [[1, hn], [0, 4]]),
                    op=ALU.subtract)
                ez = sm.tile([P, 4 * hn], F32, tag="ez")
                nc.scalar.activation(out=ez[:], in_=z[:], func=ACT.Exp)
                se = sm.tile([P, hn], F32, tag="se")
                nc.vector.tensor_reduce(
                    out=se[:], in_=ez[:].rearrange("p (t c) -> p t c", c=4),
                    axis=mybir.AxisListType.X, op=ALU.add)
                lse = sm.tile([P, hn], F32, tag="lse")
                nc.scalar.activation(out=lse[:], in_=se[:], func=ACT.Ln)
                res = sm.tile([P, 4 * hn], F32, tag="res")
                nc.vector.tensor_tensor(
                    out=res[:].rearrange("p (t c) -> p t c", c=4),
                    in0=z[:].rearrange("p (t c) -> p t c", c=4),
                    in1=_v(lse[:], 0, [[1, hn], [0, 4]]),
                    op=ALU.subtract)
                nc.sync.dma_start(
                    out=_dv(oext, t0 * P * 4, [[4, P], [P * 4, hn], [1, 4]]),
                    in_=res[:].rearrange("p (t c) -> p t c", c=4))

    nc.compile()
    return nc


def kernel(**inputs) -> np.ndarray:
    x = np.asarray(inputs["x"], np.float32)
    edge_index = np.asarray(inputs["edge_index"])
    N = x.shape[0]
    src = edge_index[0].astype(np.int64)
    dst = edge_index[1].astype(np.int64)

    plan = _plan(src, dst, N)
    T, S, N_pad = plan["T"], plan["S"], plan["N_pad"]

    consts, dum = _consts(
        np.asarray(inputs["W1"], np.float32),
        np.asarray(inputs["att_src1"], np.float32),
        np.asarray(inputs["att_dst1"], np.float32),
        np.asarray(inputs["b1"], np.float32),
        np.asarray(inputs["W2"], np.float32),
        np.asarray(inputs["att_src2"], np.float32),
        np.asarray(inputs["att_dst2"], np.float32),
        np.asarray(inputs["b2"], np.float32))

    xpad = np.zeros((N_pad, 2), np.float32)
    xpad[:N] = x
    xpad_pi = xpad[plan["perm_tab"]]

    nc = _build(T, S, plan["D"], plan["colbase"], N_pad)

    in_maps = []
    for c in range(NCORE):
        in_maps.append({
            "xpad": xpad_pi,
            "siw": plan["siw"][c],
            "xd": xpad[plan["dstid"][c]].reshape(P, -1),
            "consts": consts,
            "dum": dum,
        })

    if os.environ.get("GAT_SIM", "0") == "1":
        from concourse.bass_interp import MultiCoreSim
        sim = MultiCoreSim(nc, NCORE)
        for c in range(NCORE):
            for k, v in in_maps[c].items():
                sim.cores[c].tensor(k)[:] = v
        sim.simulate()
        outs = [np.array(sim.cores[c].tensor("out")[:]) for c in range(NCORE)]
    else:
        trace = os.environ.get("GAT_TRACE", "0") == "1"
        res = run_bass_kernel_spmd(nc, in_maps, list(range(NCORE)),
                                   trace=trace)
        if trace:
            print(f"HW exec time: {res.exec_time_ns} ns")
        outs = [res.results[c]["out"] for c in range(NCORE)]

    big = np.concatenate(outs, axis=0)
    full = np.empty((N_pad, 4), np.float32)
    full[plan["perm_tab"]] = big
    return full[:N]
